# revision 37
# baseline (speedup 1.0000x reference)
"""Trainium2 Bass kernel for batched 2D variable-coefficient diffusion CG solve.

Problem: 64 independent solves of A(alpha) u = f_rhs on a 256x256 grid,
5-point stencil with edge coefficients exp(0.5*(alpha_a + alpha_b)), solved
with 300 fp32 CG iterations (the reference's jax CG never converges before
maxiter=300 at tol=1e-6 in fp32, so the output is exactly the 300th iterate;
truncation is NOT an option: u_280 differs from u_300 by 6e-2 relative).

Sharding: pure data parallel, 8 problems per NeuronCore across 8 cores.
Per-core layout: partition P = b*16 + kb (b = local problem 0..7, kb = k-block
0..15); each partition holds 16 k-columns x 256 j-rows, free index = c*256+j
for k = kb*16 + c. All CG state lives in SBUF for all 300 iterations.

Default variant "v3" (16.7ms total, 52-56us/iter, rel err 1.5446e-2):
pipelined CG with the q-recurrence q_{k+1} = A r_{k+1} + beta_k q_k, so the
stencil runs on r right after the r-update while the beta/p chain hides
behind it, and a FLUX-FORM stencil: Fj = cl*(u_j - u_{j-1}), q = Fj_j -
Fj_{j+1} + G_k - G_{k+1}, which matches the reference operator exactly
(boundary masks folded into zero coefficients; the j==0 Dirichlet double
term lands via a tiny [128,16] fix pass). 7 big passes for the stencil vs
9 for the product form.

Engine split, driven by measured HW behavior (see exp/ + project memory):
ANY GpSimd op taxes the whole kernel ~6us via shared SBUF ports, so the
steady-state loop uses NO GpSimd at all. DVE runs the whole elementwise
chain (12 passes: dj, e, Fj, a1, t0, G, s2, q-join, x-tail, p, pAp, r);
ACT computes ||r||^2 (Square+accum_out), refreshes the SBUF halo columns
from PSUM, and builds diag(alpha); PE does the two halo shift matmuls, the
per-problem segment-sum/broadcast reductions (block-diagonal ones-matmul),
and accumulates x[:, 0:3072] = sum alpha_k p_k into six persistent PSUM
banks via diag(alpha_{k-1}) matmuls (LAGGED one iteration so they never
wait on alpha; p is double-buffered by body parity so those reads never
race the p update). hhi/pap/gam share the last PSUM bank (every overwrite
lands after the previous tenant's consumer). The x tail (1024 cols) is a
DVE STT. Emission order IS Tile program order: consumers emitted before
their producer bind to the previous iteration (used deliberately for the
lagged x and the stale-halo dj read, which cl[0]=0 kills).

Numerics: the fp32 trajectory wanders ~1.5e-2 relative between equally
valid implementations (gate 2e-2); any arithmetic reordering rerolls that
sample, so changes must be re-verified with test.py (deterministic seed).
x-accumulation order does not affect the trajectory (x never feeds back).

Measured-and-reverted: 2D->3D AP rewrite of the std variant (81.7us),
STT-with-immediate-*1.0 everywhere (97.4), strided (257-stride) GpSimd
e/p ops (v2, 76.2), GpSimd contiguous e/G/p/x (v3-pool, 65.2). Loop is
For_i unrolled x4 (unroll2/8 measured worse). Timing method: iteration-
count slope (300 vs 2400) isolates NEFF execution from ~230-260ms fixed
axon dispatch; run-to-run variance is +-3us/iter between processes.
"""

import os
import numpy as np

M = 256
B = 64
NCORES = 8
BPC = B // NCORES          # problems per core
HINV2 = np.float32(M * M)  # exact power of two: folding into coeffs is exact
ITERS = 300
COLS = 16                  # k-columns per partition
F = COLS * M               # 4096 free elements per field
FH = F + 2 * M             # p buffer with halo columns

_CACHE = {}


# ----------------------------------------------------------------- host side

def _coeff_arrays(alpha):
    """Per-problem stencil coefficient fields, matching reference._stencil_coeffs
    fp32 op-for-op, with HINV2 folded in (exact) and off-diagonals negated.

    alpha: (B, 257, 257) f32. Returns diag, KL, KB as (B, 256, 256) f32 where
    KL/KB are the *unmasked-left* / *masked-bottom* edge coefficients."""
    a = alpha.astype(np.float32)
    m = M
    j = np.arange(m)[:, None]
    k = np.arange(m)[None, :]
    KL = np.exp(np.float32(0.5) * (a[:, :-1, :-1] + a[:, :-1, 1:])).astype(np.float32)
    KR = np.where(j < m - 1,
                  np.exp(np.float32(0.5) * (a[:, 1:, :-1] + a[:, 1:, 1:])),
                  np.float32(0.0)).astype(np.float32)
    KB = np.where(k > 0,
                  np.exp(np.float32(0.5) * (a[:, :-1, :-1] + a[:, 1:, :-1])),
                  np.float32(0.0)).astype(np.float32)
    KT = np.where(k < m - 1,
                  np.exp(np.float32(0.5) * (a[:, :-1, 1:] + a[:, 1:, 1:])),
                  np.float32(0.0)).astype(np.float32)
    diag = KL + KR + KB + KT + np.where(j == 0, KL, np.float32(0.0)).astype(np.float32)
    return diag, KL, KB


def _to_dev(arr_bjk):
    """(BPC, 256j, 256k) -> [128, 4096] with P = b*16+kb, free = c*256+j."""
    t = arr_bjk.transpose(0, 2, 1)                 # (b, k, j)
    t = t.reshape(BPC, 16, COLS, M)                # (b, kb, c, j)
    return np.ascontiguousarray(t.reshape(128, F))


def _from_dev(dev):
    """[128, 4096] -> (BPC, 256j, 256k)."""
    t = dev.reshape(BPC, 16, COLS, M).transpose(0, 3, 1, 2)   # (b, j, kb, c)
    return np.ascontiguousarray(t.reshape(BPC, M, M))


def _pack_core(alpha_core, f_rhs):
    """Build the per-core input map (all fp32 numpy arrays)."""
    diag, KL, KB = _coeff_arrays(alpha_core)
    s = HINV2
    cD = _to_dev(diag * s)                               # [128, 4096]
    nKL = _to_dev(KL * (-s)).reshape(128, COLS, M)       # (P, c, j)
    nKB = _to_dev(KB * (-s)).reshape(128, COLS, M)

    # cLp[P, c, 0..256]: 0 at jj=0 (Dirichlet kill for the j-1 shift),
    # -s*KL[jj,k] at jj=1..255, 0 at jj=256 (K_right mask at j=255).
    cLp = np.zeros((128, COLS, M + 1), np.float32)
    cLp[:, :, 1:M] = nKL[:, :, 1:M]

    # cBp[P, 0..16, j]: c=0..15 the (already k-masked) bottom coefficients,
    # c=16 the next partition's c=0 column (static k-halo; 0 past k=255).
    cBp = np.zeros((128, COLS + 1, M), np.float32)
    cBp[:, :COLS, :] = nKB
    nKB4 = nKB.reshape(BPC, 16, COLS, M)
    cBp4 = cBp.reshape(BPC, 16, COLS + 1, M)
    cBp4[:, :-1, COLS, :] = nKB4[:, 1:, 0, :]

    fdev = _to_dev(np.broadcast_to(f_rhs, (BPC, M, M)).astype(np.float32))

    seg = np.zeros((128, BPC), np.float32)               # seg[q, b] = q//16 == b
    seg[np.arange(128), np.arange(128) // 16] = 1.0
    bc = np.ascontiguousarray(seg.T)                     # (8, 128)
    qi = np.arange(128)
    bc128 = (qi[:, None] // 16 == qi[None, :] // 16).astype(np.float32)
    sdn = np.eye(128, 128, 1, np.float32)                # out[i] = in[i-1]
    sup = np.eye(128, 128, -1, np.float32)               # out[i] = in[i+1]

    return {
        "f_in": fdev,
        "cD_in": cD,
        "cL_in": np.ascontiguousarray(cLp.reshape(128, COLS * (M + 1))),
        "cB_in": np.ascontiguousarray(cBp.reshape(128, (COLS + 1) * M)),
        "seg_in": seg,
        "bc_in": bc,
        "bc128_in": bc128,
        "sdn_in": sdn,
        "sup_in": sup,
    }


CS2 = M + 1                 # 257: column stride in the padded r buffer
RBW = 1 + COLS * CS2        # 4113: [pre-pad | 16 x (256 data + 1 pad)]
XS = 2048                   # trailing x columns accumulated on GpSimd (rest in PSUM)


def _pack_core_v2(alpha_core, f_rhs):
    """Inputs for the flux-form q-recurrence kernel (_build_nc_v2).

    cL2[P, c, jj] (jj=0..256) multiplies dj[jj] = u[jj]-u[jj-1] (ghost 0):
      jj=0 -> 2*s*KL[0] (the j==0 Dirichlet extra folds in exactly),
      jj=1..255 -> s*KL[jj], jj=256 -> 0 (K_right mask at j=255).
    cB2[P, c', j] (c'=0..16) multiplies e[c'] = u[col c']-u[col c'-1]:
      s*K_bottom at k=kb*16+c' (0 at k=0 and k=256 kills halo garbage).
    """
    _, KL, KB = _coeff_arrays(alpha_core)
    s = HINV2
    KLd = (KL * s).astype(np.float32)               # (b, j, k); *s exact (2^16)
    KLd[:, 0, :] *= np.float32(2.0)
    t = KLd.transpose(0, 2, 1).reshape(BPC, 16, COLS, M)   # (b, kb, c, j)
    cL2 = np.zeros((BPC, 16, COLS, CS2), np.float32)
    cL2[..., :M] = t
    cL2 = np.ascontiguousarray(cL2.reshape(128, COLS * CS2))

    KBe = np.concatenate([(KB * s).astype(np.float32),
                          np.zeros((BPC, M, 1), np.float32)], axis=2)  # (b, j, 257k)
    tb = KBe.transpose(0, 2, 1)                     # (b, k, j)
    idx = np.arange(16)[:, None] * COLS + np.arange(COLS + 1)[None, :]
    cB2 = tb[:, idx, :]                             # (b, kb, 17, j)
    cB2 = np.ascontiguousarray(cB2.reshape(128, (COLS + 1) * M))

    fdev = _to_dev(np.broadcast_to(f_rhs, (BPC, M, M)).astype(np.float32))
    qi = np.arange(128)
    bc128 = (qi[:, None] // 16 == qi[None, :] // 16).astype(np.float32)
    return {
        "f_in": fdev,
        "cL2_in": cL2,
        "cB2_in": cB2,
        "bc128_in": bc128,
        "sdn_in": np.eye(128, 128, 1, np.float32),
        "sup_in": np.eye(128, 128, -1, np.float32),
        "eye_in": np.eye(128, dtype=np.float32),
    }


def _build_nc_v2(iters):
    """Flux-form pipelined-CG (q-recurrence) kernel.

    Loop boundary sits after the r-update: body k computes gamma_k/beta_{k-1}
    (ACT rr + PE segment-sum), the flux stencil w = A r_k (j-chain on DVE,
    e/G on GpSimd with halo columns read straight from PSUM), the p/q
    recurrences, pAp_k/alpha_k, and r_{k+1}. The x-accumulation is LAGGED one
    iteration (x += alpha_{k-1} p_{k-1}) so it never waits on alpha: PE
    accumulates columns 0:3072 into six persistent PSUM banks via a
    diag(alpha) matmul, GpSimd STTs the last 1024 columns. p is double
    buffered (body parity) so those reads never race the p update.
    """
    from contextlib import ExitStack
    import concourse.tile as tile
    from concourse import bacc, mybir

    f32 = mybir.dt.float32
    Alu = mybir.AluOpType
    Act = mybir.ActivationFunctionType

    nc = bacc.Bacc("TRN2", target_bir_lowering=False, debug=False)

    f_d = nc.dram_tensor("f_in", [128, F], f32, kind="ExternalInput").ap()
    cL_d = nc.dram_tensor("cL2_in", [128, COLS * CS2], f32, kind="ExternalInput").ap()
    cB_d = nc.dram_tensor("cB2_in", [128, (COLS + 1) * M], f32, kind="ExternalInput").ap()
    bc128_d = nc.dram_tensor("bc128_in", [128, 128], f32, kind="ExternalInput").ap()
    sdn_d = nc.dram_tensor("sdn_in", [128, 128], f32, kind="ExternalInput").ap()
    sup_d = nc.dram_tensor("sup_in", [128, 128], f32, kind="ExternalInput").ap()
    eye_d = nc.dram_tensor("eye_in", [128, 128], f32, kind="ExternalInput").ap()
    x_d = nc.dram_tensor("x_out", [128, F], f32, kind="ExternalOutput").ap()

    with tile.TileContext(nc) as tc, ExitStack() as ctx:
        sb = ctx.enter_context(tc.tile_pool(name="state", bufs=1))
        ps = ctx.enter_context(tc.tile_pool(name="psum", bufs=1, space="PSUM"))

        rb = sb.tile([128, RBW], f32, name="rb")
        pA = sb.tile([128, F], f32, name="pA")
        pB = sb.tile([128, F], f32, name="pB")
        q = sb.tile([128, F], f32, name="q")
        dj = sb.tile([128, COLS * CS2], f32, name="dj")    # then Fj in place
        a1 = sb.tile([128, F], f32, name="a1")             # + pAp junk out
        t0 = sb.tile([128, F], f32, name="t0")             # + ACT rr junk out
        tp = sb.tile([128, F], f32, name="tp")             # beta*p_prev (ACT)
        tx = sb.tile([128, XS], f32, name="tx")            # alpha_prev*p tail
        eG = sb.tile([128, (COLS + 1) * M], f32, name="eG")  # e then G in place
        xsb = sb.tile([128, XS], f32, name="xsb")
        cL = sb.tile([128, COLS * CS2], f32, name="cL")
        cB = sb.tile([128, (COLS + 1) * M], f32, name="cB")
        eye = sb.tile([128, 128], f32, name="eye")
        bc128 = sb.tile([128, 128], f32, name="bc128")
        sdn = sb.tile([128, 128], f32, name="sdn")
        sup = sb.tile([128, 128], f32, name="sup")
        diag = sb.tile([128, 128], f32, name="diag")

        rr_part = sb.tile([128, 1], f32, name="rr_part")
        pap_part = sb.tile([128, 1], f32, name="pap_part")
        gamvec = sb.tile([128, 1], f32, name="gamvec")
        recg = sb.tile([128, 1], f32, name="recg")
        recp = sb.tile([128, 1], f32, name="recp")
        avec = sb.tile([128, 1], f32, name="avec")
        avp = sb.tile([128, 1], f32, name="avp")    # alpha_{k-1} for lagged x
        aneg = sb.tile([128, 1], f32, name="aneg")
        bvec = sb.tile([128, 1], f32, name="bvec")

        xps = [ps.tile([128, 512], f32, name=f"xps{j}") for j in range(4)]
        hlo_ps = ps.tile([128, M], f32, name="hlo_ps")
        hhi_ps = ps.tile([128, M], f32, name="hhi_ps")
        pap_ps = ps.tile([128, 1], f32, name="pap_ps")
        gam_ps = ps.tile([128, 1], f32, name="gam_ps")

        # ---- views over the padded r buffer
        rb2 = rb[:]
        def cj(ap2d, c, j):
            return ap2d.rearrange("p (c j) -> p c j", c=c, j=j)
        r_c3 = cj(rb2[:, 1:1 + COLS * CS2], COLS, CS2)[:, :, 0:M]
        dj_in0 = cj(rb2[:, 1:1 + COLS * CS2], COLS, CS2)
        dj_in1 = cj(rb2[:, 0:COLS * CS2], COLS, CS2)
        col0 = rb2[:, 1:1 + M]
        col15 = rb2[:, 1 + 15 * CS2:1 + 15 * CS2 + M]
        em_out = cj(eG[:, M:M + 15 * M], 15, M)
        em_in0 = cj(rb2[:, 1 + CS2:1 + 16 * CS2], 15, CS2)[:, :, 0:M]
        em_in1 = cj(rb2[:, 1:1 + 15 * CS2], 15, CS2)[:, :, 0:M]
        e_lo = eG[:, 0:M]
        e_hi = eG[:, COLS * M:COLS * M + M]
        G_lo = eG[:, 0:F]
        G_hi = eG[:, M:M + F]
        dj3 = cj(dj[:], COLS, CS2)
        Fj_lo = dj3[:, :, 0:M]
        Fj_hi = dj3[:, :, 1:CS2]

        # ---- init
        nc.vector.memset(rb[:], 0.0)
        nc.vector.memset(pA[:], 0.0)
        nc.vector.memset(pB[:], 0.0)
        nc.vector.memset(q[:], 0.0)
        nc.vector.memset(xsb[:], 0.0)
        nc.vector.memset(diag[:], 0.0)
        nc.vector.memset(avec[:], 0.0)
        nc.vector.memset(avp[:], 0.0)
        nc.vector.memset(recg[:], 0.0)
        nc.vector.memset(gamvec[:], 1.0)
        for j in range(4):
            nc.vector.memset(xps[j][:], 0.0)
        nc.sync.dma_start(cL[:], cL_d)
        nc.sync.dma_start(cB[:], cB_d)
        nc.sync.dma_start(bc128[:], bc128_d)
        nc.sync.dma_start(sdn[:], sdn_d)
        nc.sync.dma_start(sup[:], sup_d)
        nc.sync.dma_start(eye[:], eye_d)
        nc.sync.dma_start(r_c3, f_d)

        cnt = [0]

        GS = 9 * M      # DVE handles G over e-cols 0..8, Pool cols 9..16
        ablate = set(os.environ.get("KERNEL2_ABLATE", "").split(","))

        def d3(ap2d):
            return ap2d.rearrange("p (c j) -> p c j", c=COLS, j=M)

        def body(_iv):
            k = cnt[0]
            cnt[0] += 1
            p_cur = pA if k % 2 == 0 else pB
            p_prev = pB if k % 2 == 0 else pA

            # Emission order IS program order: every consumer must be emitted
            # after the producer whose value it wants (per-engine queues then
            # run in-order, so off-path waits don't block earlier ops).

            # gamma_k = ||r_k||^2 (ACT), then its segment-sum (PE)
            if "rr" not in ablate:
                nc.scalar.activation(d3(t0[:]), r_c3, Act.Square, accum_out=rr_part[:])
            nc.tensor.matmul(hlo_ps[:], sdn[:], col15)
            nc.tensor.matmul(hhi_ps, sup[:], col0)
            nc.tensor.matmul(gam_ps[:], bc128[:], rr_part[:])
            if "xmm" not in ablate:
                for j in range(4):
                    nc.tensor.matmul(xps[j][:], diag[:], p_prev[:, j * 512:(j + 1) * 512],
                                     start=False, stop=False, skip_group_check=True)

            # DVE head: dj (no deps), halo-edge e cols, beta_{k-1}
            nc.vector.tensor_tensor(cj(dj[:], COLS, CS2), dj_in0, dj_in1,
                                    Alu.subtract)
            nc.vector.tensor_tensor(e_lo, col0, hlo_ps[:], Alu.subtract)
            nc.vector.tensor_tensor(e_hi, hhi_ps, col15, Alu.subtract)
            nc.vector.tensor_mul(bvec[:], gam_ps, recg[:])   # beta = gam_k/gam_{k-1}

            # ACT: gamma copy; beta*p_prev; alpha_{k-1}*p_prev tail
            nc.scalar.copy(gamvec[:], gam_ps[:])
            if "actp" not in ablate:
                nc.scalar.activation(tp[:], p_prev[:], Act.Copy, scale=bvec[:])
                nc.scalar.activation(tx[:], p_prev[:, F - XS:F], Act.Copy, scale=avp[:])

            # GpSimd: e interior, G tail, p update, x tail
            if "poole" not in ablate:
                nc.gpsimd.tensor_tensor(em_out, em_in0, em_in1, Alu.subtract)
                nc.gpsimd.tensor_tensor(eG[:, GS:], cB[:, GS:], eG[:, GS:], Alu.mult)
            if "poolp" not in ablate:
                nc.gpsimd.tensor_tensor(d3(p_cur[:]), d3(tp[:]), r_c3, Alu.add)
                nc.gpsimd.tensor_tensor(xsb[:], xsb[:], tx[:], Alu.add)

            # DVE tail: j-flux, G head, q recurrence, pAp, alpha, r-update
            nc.vector.tensor_mul(dj[:], cL[:], dj[:])                  # Fj
            nc.vector.tensor_tensor(d3(a1[:]), Fj_lo, Fj_hi, Alu.subtract)
            nc.vector.scalar_tensor_tensor(
                t0[:], q[:], bvec[:], a1[:], Alu.mult, Alu.add)
            nc.vector.tensor_mul(eG[:, 0:GS], cB[:, 0:GS], eG[:, 0:GS])  # G head
            nc.vector.tensor_add(t0[:], t0[:], G_lo)
            nc.vector.tensor_tensor(q[:], t0[:], G_hi, Alu.subtract)   # q_k
            nc.vector.scalar_tensor_tensor(
                a1[:], p_cur[:], 1.0, q[:], Alu.mult, Alu.mult,
                accum_out=pap_part[:])
            nc.tensor.matmul(pap_ps, bc128[:], pap_part[:])
            nc.vector.reciprocal(recp[:], pap_ps)
            nc.vector.tensor_mul(avec[:], gamvec[:], recp[:])
            nc.vector.tensor_scalar_mul(aneg[:], avec[:], -1.0)
            nc.vector.reciprocal(recg[:], gamvec[:])     # 1/gamma_k for body k+1
            nc.vector.scalar_tensor_tensor(
                r_c3, d3(q[:]), aneg[:], r_c3, Alu.mult, Alu.add)      # r_{k+1}

            # ACT tail: alpha_k staged for next body's lagged x
            nc.scalar.activation(diag[:], eye[:], Act.Copy, scale=avec[:])
            nc.scalar.copy(avp[:], avec[:])

        loop_mode = os.environ.get("KERNEL2_LOOP", "unroll4")
        if loop_mode == "py":
            for _i in range(iters):
                body(_i)
        else:
            tc.For_i_unrolled(0, iters, 1, body, max_unroll=int(loop_mode[6:]))

        # epilogue: the x contribution of the last body, then readout
        p_last = pA if (iters - 1) % 2 == 0 else pB
        for j in range(6):
            nc.tensor.matmul(xps[j][:], diag[:], p_last[:, j * 512:(j + 1) * 512],
                             start=False, stop=True, skip_group_check=True)
        nc.scalar.activation(tx[:], p_last[:, F - XS:F], Act.Copy, scale=avp[:])
        nc.gpsimd.tensor_tensor(xsb[:], xsb[:], tx[:], Alu.add)
        for j in range(4):
            nc.scalar.copy(dj[:, j * 512:(j + 1) * 512], xps[j][:])
        nc.sync.dma_start(x_d[:, 0:4 * 512], dj[:, 0:4 * 512])
        nc.sync.dma_start(x_d[:, 4 * 512:F], xsb[:])

        if os.environ.get("KERNEL2_DEBUG"):
            dbg = {}
            for nm, width in [("q_dbg", F), ("rb_dbg", RBW), ("eG_dbg", (COLS + 1) * M),
                              ("a1_dbg", F), ("pA_dbg", F), ("pB_dbg", F),
                              ("gam_dbg", 1), ("avec_dbg", 1), ("bvec_dbg", 1),
                              ("pap_dbg", 1), ("rr_dbg", 1)]:
                dbg[nm] = nc.dram_tensor(nm, [128, width], f32,
                                         kind="ExternalOutput").ap()
            nc.sync.dma_start(dbg["q_dbg"], q[:])
            nc.sync.dma_start(dbg["rb_dbg"], rb[:])
            nc.sync.dma_start(dbg["eG_dbg"], eG[:])
            nc.sync.dma_start(dbg["a1_dbg"], a1[:])
            nc.sync.dma_start(dbg["pA_dbg"], pA[:])
            nc.sync.dma_start(dbg["pB_dbg"], pB[:])
            nc.sync.dma_start(dbg["gam_dbg"], gamvec[:])
            nc.sync.dma_start(dbg["avec_dbg"], avec[:])
            nc.sync.dma_start(dbg["bvec_dbg"], bvec[:])
            nc.sync.dma_start(dbg["pap_dbg"], pap_part[:])
            nc.sync.dma_start(dbg["rr_dbg"], rr_part[:])

    nc.compile()
    return nc


def _pack_core_v3(alpha_core, f_rhs):
    """Inputs for the contiguous flux-form q-recurrence kernel (_build_nc_v3).

    cl[P, c*256+jj]: 0 at jj=0 (kills the cross-column garbage difference),
    s*KL[jj,k] for jj=1..255. The j==0 flux (with its Dirichlet doubling)
    is applied by a tiny [128,16] fix pass using c0[P,c] = 2*s*KL[0,k].
    cB2 is the same 17-column k-flux coefficient set as v2.
    """
    _, KL, KB = _coeff_arrays(alpha_core)
    s = HINV2
    KLs = (KL * s).astype(np.float32)               # (b, j, k)
    t = KLs.transpose(0, 2, 1).reshape(BPC, 16, COLS, M)   # (b, kb, c, j)
    cl = t.copy()
    cl[..., 0] = np.float32(0.0)
    cl = np.ascontiguousarray(cl.reshape(128, F))
    c0 = np.ascontiguousarray(
        (2.0 * t[..., 0]).astype(np.float32).reshape(128, COLS))

    KBe = np.concatenate([(KB * s).astype(np.float32),
                          np.zeros((BPC, M, 1), np.float32)], axis=2)
    tb = KBe.transpose(0, 2, 1)
    idx = np.arange(16)[:, None] * COLS + np.arange(COLS + 1)[None, :]
    cB2 = np.ascontiguousarray(tb[:, idx, :].reshape(128, (COLS + 1) * M))

    fdev = _to_dev(np.broadcast_to(f_rhs, (BPC, M, M)).astype(np.float32))
    qi = np.arange(128)
    bc128 = (qi[:, None] // 16 == qi[None, :] // 16).astype(np.float32)
    return {
        "f_in": fdev,
        "cl_in": cl,
        "c0_in": c0,
        "cB2_in": cB2,
        "bc128_in": bc128,
        "sdn_in": np.eye(128, 128, 1, np.float32),
        "sup_in": np.eye(128, 128, -1, np.float32),
        "eye_in": np.eye(128, dtype=np.float32),
    }


def _build_nc_v3(iters):
    """Contiguous-layout flux-form pipelined CG (see _build_nc_v2 docstring).

    Differences from v2: r lives in a plain contiguous [128, 1+4096] buffer
    (one leading pad for the j-shift), Fj gets a zeroed tail slot so the
    j-flux difference needs no column pads, e/G/a2 are contiguous slices,
    and the j==0 flux lands via two tiny [128,16] strided ops. This removes
    every big strided access pattern (strided DVE passes measured +2us each).
    """
    from contextlib import ExitStack
    import concourse.tile as tile
    from concourse import bacc, mybir

    f32 = mybir.dt.float32
    Alu = mybir.AluOpType
    Act = mybir.ActivationFunctionType

    nc = bacc.Bacc("TRN2", target_bir_lowering=False, debug=False)

    f_d = nc.dram_tensor("f_in", [128, F], f32, kind="ExternalInput").ap()
    cl_d = nc.dram_tensor("cl_in", [128, F], f32, kind="ExternalInput").ap()
    c0_d = nc.dram_tensor("c0_in", [128, COLS], f32, kind="ExternalInput").ap()
    cB_d = nc.dram_tensor("cB2_in", [128, (COLS + 1) * M], f32, kind="ExternalInput").ap()
    bc128_d = nc.dram_tensor("bc128_in", [128, 128], f32, kind="ExternalInput").ap()
    sdn_d = nc.dram_tensor("sdn_in", [128, 128], f32, kind="ExternalInput").ap()
    sup_d = nc.dram_tensor("sup_in", [128, 128], f32, kind="ExternalInput").ap()
    eye_d = nc.dram_tensor("eye_in", [128, 128], f32, kind="ExternalInput").ap()
    x_d = nc.dram_tensor("x_out", [128, F], f32, kind="ExternalOutput").ap()

    with tile.TileContext(nc) as tc, ExitStack() as ctx:
        sb = ctx.enter_context(tc.tile_pool(name="state", bufs=1))
        ps = ctx.enter_context(tc.tile_pool(name="psum", bufs=1, space="PSUM"))

        rp = sb.tile([128, 2 * M + F], f32, name="rp")  # [halo_lo | r | halo_hi]
        pA = sb.tile([128, F], f32, name="pA")
        pB = sb.tile([128, F], f32, name="pB")
        q = sb.tile([128, F], f32, name="q")
        djF = sb.tile([128, F + 1], f32, name="djF")   # dj then Fj; tail slot 0
        a1 = sb.tile([128, F], f32, name="a1")         # + pAp junk out
        t0 = sb.tile([128, F], f32, name="t0")         # + ACT rr junk out
        tp = sb.tile([128, F], f32, name="tp")
        tx = sb.tile([128, XS], f32, name="tx")
        t16 = sb.tile([128, COLS], f32, name="t16")
        eG = sb.tile([128, (COLS + 1) * M], f32, name="eG")
        xsb = sb.tile([128, XS], f32, name="xsb")
        cl = sb.tile([128, F], f32, name="cl")
        c0 = sb.tile([128, COLS], f32, name="c0")
        cB = sb.tile([128, (COLS + 1) * M], f32, name="cB")
        eye = sb.tile([128, 128], f32, name="eye")
        bc128 = sb.tile([128, 128], f32, name="bc128")
        sdn = sb.tile([128, 128], f32, name="sdn")
        sup = sb.tile([128, 128], f32, name="sup")
        diag = sb.tile([128, 128], f32, name="diag")

        rr_part = sb.tile([128, 1], f32, name="rr_part")
        pap_part = sb.tile([128, 1], f32, name="pap_part")
        gamvec = sb.tile([128, 1], f32, name="gamvec")
        recg = sb.tile([128, 1], f32, name="recg")
        recp = sb.tile([128, 1], f32, name="recp")
        avec = sb.tile([128, 1], f32, name="avec")
        avp = sb.tile([128, 1], f32, name="avp")
        aneg = sb.tile([128, 1], f32, name="aneg")
        bvec = sb.tile([128, 1], f32, name="bvec")

        xps = [ps.tile([128, 512], f32, name=f"xps{j}") for j in range(6)]
        hlo_ps = ps.tile([128, M], f32, name="hlo_ps")
        # hhi + pap + gam share the last bank: each overwrite happens after
        # the previous tenant's consumer has read (hhi->e_hi early, gam->bvec
        # early, pap->recp late), so whole-bank matmul resets are harmless.
        hb = ps.tile([128, M + 2], f32, name="hb")
        hhi_ps = hb[:, 0:M]
        pap_ps = hb[:, M:M + 1]
        gam_ps = hb[:, M + 1:M + 2]

        r = rp[:, M:M + F]                      # contiguous center
        col0 = rp[:, M:2 * M]
        col15 = rp[:, F:F + M]
        dj_in0 = rp[:, M:M + F]
        dj_in1 = rp[:, M - 1:M - 1 + F]         # halo_lo[255] killed by cl[0]=0
        Fj_lo = djF[:, 0:F]
        Fj_hi = djF[:, 1:1 + F]
        e_in0 = rp[:, M:M + F + M]              # cols 0..15 | halo_hi
        e_in1 = rp[:, 0:F + M]                  # halo_lo | cols 0..15
        h_lo = rp[:, 0:M]
        h_hi = rp[:, F + M:F + 2 * M]
        G_lo = eG[:, 0:F]
        G_hi = eG[:, M:M + F]
        # j==0 positions of r / a1: stride-256 scatter of 16 per partition
        r_j0 = r.rearrange("p (c j) -> p c j", c=COLS, j=M)[:, :, 0:1]
        a1_j0 = a1[:].rearrange("p (c j) -> p c j", c=COLS, j=M)[:, :, 0:1]
        t16v = t16[:].rearrange("p (c o) -> p c o", c=COLS, o=1)

        # ---- init
        nc.vector.memset(rp[:], 0.0)
        nc.vector.memset(djF[:], 0.0)
        nc.vector.memset(pA[:], 0.0)
        nc.vector.memset(pB[:], 0.0)
        nc.vector.memset(q[:], 0.0)
        nc.vector.memset(xsb[:], 0.0)
        nc.vector.memset(diag[:], 0.0)
        nc.vector.memset(avec[:], 0.0)
        nc.vector.memset(avp[:], 0.0)
        nc.vector.memset(recg[:], 0.0)
        nc.vector.memset(gamvec[:], 1.0)
        for j in range(6):
            nc.vector.memset(xps[j][:], 0.0)
        nc.sync.dma_start(cl[:], cl_d)
        nc.sync.dma_start(c0[:], c0_d)
        nc.sync.dma_start(cB[:], cB_d)
        nc.sync.dma_start(bc128[:], bc128_d)
        nc.sync.dma_start(sdn[:], sdn_d)
        nc.sync.dma_start(sup[:], sup_d)
        nc.sync.dma_start(eye[:], eye_d)
        nc.sync.dma_start(r, f_d)

        GS = 9 * M
        ablate = set(os.environ.get("KERNEL2_ABLATE", "").split(","))
        dve_mode = os.environ.get("KERNEL3_MODE", "dve") == "dve"
        cnt = [0]

        def body(_iv):
            k = cnt[0]
            cnt[0] += 1
            p_cur = pA if k % 2 == 0 else pB
            p_prev = pB if k % 2 == 0 else pA

            # dj first: its single halo_lo[255] read is killed by cl[0]=0, so
            # emitting it before the halo refresh binds it to the stale halo
            # and keeps DVE from waiting on the copies.
            nc.vector.tensor_tensor(djF[:, 0:F], dj_in0, dj_in1, Alu.subtract)

            # halo shift matmuls (PE) -> SBUF halo slots (ACT copies), then
            # gamma_k = ||r_k||^2 (ACT) -> segment sum (PE); lagged x matmuls
            nc.tensor.matmul(hlo_ps[:], sdn[:], col15)
            nc.tensor.matmul(hhi_ps, sup[:], col0)
            nc.scalar.copy(h_lo, hlo_ps[:])
            nc.scalar.copy(h_hi, hhi_ps)
            nc.scalar.activation(t0[:], r, Act.Square, accum_out=rr_part[:])
            nc.tensor.matmul(gam_ps, bc128[:], rr_part[:])
            if "xmm" not in ablate:
                for j in range(6):
                    nc.tensor.matmul(xps[j][:], diag[:], p_prev[:, j * 512:(j + 1) * 512],
                                     start=False, stop=False, skip_group_check=True)

            # DVE: e in one contiguous diff over the halo'd buffer; j0 flux;
            # beta_{k-1}
            nc.vector.tensor_tensor(eG[:], e_in0, e_in1, Alu.subtract)
            nc.vector.tensor_tensor(t16v, c0[:].rearrange("p (c o) -> p c o", c=COLS, o=1),
                                    r_j0, Alu.mult)
            nc.vector.tensor_mul(bvec[:], gam_ps, recg[:])

            # ACT: gamma copy
            nc.scalar.copy(gamvec[:], gam_ps)

            # DVE tail: Fj, a1 (+j0 fix), q recurrence, x/p, pAp, alpha, r
            nc.vector.tensor_mul(djF[:, 0:F], cl[:], djF[:, 0:F])     # Fj
            nc.vector.tensor_tensor(a1[:], Fj_lo, Fj_hi, Alu.subtract)
            nc.vector.tensor_tensor(a1_j0, a1_j0, t16v, Alu.add)      # j0 flux
            nc.vector.scalar_tensor_tensor(
                t0[:], q[:], bvec[:], a1[:], Alu.mult, Alu.add)
            nc.vector.tensor_mul(eG[:], cB[:], eG[:])                 # G
            nc.vector.tensor_add(t0[:], t0[:], G_lo)
            nc.vector.tensor_tensor(q[:], t0[:], G_hi, Alu.subtract)  # q_k
            nc.vector.scalar_tensor_tensor(
                xsb[:, 0:F - 6 * 512], p_prev[:, 6 * 512:F], avp[:],
                xsb[:, 0:F - 6 * 512], Alu.mult, Alu.add)
            nc.vector.scalar_tensor_tensor(
                p_cur[:], p_prev[:], bvec[:], r, Alu.mult, Alu.add)
            nc.vector.scalar_tensor_tensor(
                a1[:], p_cur[:], 1.0, q[:], Alu.mult, Alu.mult,
                accum_out=pap_part[:])
            nc.tensor.matmul(pap_ps, bc128[:], pap_part[:])
            nc.vector.reciprocal(recp[:], pap_ps)
            nc.vector.tensor_mul(avec[:], gamvec[:], recp[:])
            nc.vector.tensor_scalar_mul(aneg[:], avec[:], -1.0)
            nc.vector.reciprocal(recg[:], gamvec[:])
            nc.vector.scalar_tensor_tensor(
                r, q[:], aneg[:], r, Alu.mult, Alu.add)               # r_{k+1}

            # ACT tail: alpha_k staged for next body's lagged x
            nc.scalar.activation(diag[:], eye[:], Act.Copy, scale=avec[:])
            nc.scalar.copy(avp[:], avec[:])

        loop_mode = os.environ.get("KERNEL2_LOOP", "unroll4")
        if loop_mode == "py":
            for _i in range(iters):
                body(_i)
        else:
            tc.For_i_unrolled(0, iters, 1, body, max_unroll=int(loop_mode[6:]))

        # epilogue: last body's x contribution, then readout
        p_last = pA if (iters - 1) % 2 == 0 else pB
        for j in range(6):
            nc.tensor.matmul(xps[j][:], diag[:], p_last[:, j * 512:(j + 1) * 512],
                             start=False, stop=True, skip_group_check=True)
        if os.environ.get("KERNEL3_MODE", "dve") == "dve":
            nc.vector.scalar_tensor_tensor(
                xsb[:, 0:F - 6 * 512], p_last[:, 6 * 512:F], avp[:],
                xsb[:, 0:F - 6 * 512], Alu.mult, Alu.add)
        else:
            nc.scalar.activation(tx[:], p_last[:, F - XS:F], Act.Copy, scale=avp[:])
            nc.gpsimd.tensor_tensor(xsb[:], xsb[:], tx[:], Alu.add)
        for j in range(6):
            nc.scalar.copy(djF[:, j * 512:(j + 1) * 512], xps[j][:])
        nc.sync.dma_start(x_d[:, 0:6 * 512], djF[:, 0:6 * 512])
        nc.sync.dma_start(x_d[:, 6 * 512:F], xsb[:, 0:F - 6 * 512])

    nc.compile()
    return nc


# --------------------------------------------------------------- bass kernel

def _build_nc_qrec(iters):
    """q-recurrence variant: q_{k+1} = A r_{k+1} + beta_k q_k.

    The stencil runs on r (available right after the r update), so the
    ||r||^2 / beta / p-update chain hides behind it. Validated in exp3.py:
    lands as close to the f64 trajectory as plain fp32 CG.

    Loop state: p, q (= A p), r (halo'd), x, gamvec ([128,1] per-problem
    gamma broadcast). Body:
        pAp = <p, q>; alpha = gamma/pAp
        x += alpha p ; r -= alpha q ; refresh r halos
        gamma' = ||r||^2 ; beta = gamma'/gamma
        w = A r  (overlaps beta chain and p update)
        p = r + beta p ; q = w + beta q
    """
    from contextlib import ExitStack
    import concourse.bass as bass
    import concourse.tile as tile
    from concourse import bacc, mybir

    f32 = mybir.dt.float32
    Alu = mybir.AluOpType
    Act = mybir.ActivationFunctionType

    nc = bacc.Bacc("TRN2", target_bir_lowering=False, debug=False)

    f_d = nc.dram_tensor("f_in", [128, F], f32, kind="ExternalInput").ap()
    cD_d = nc.dram_tensor("cD_in", [128, F], f32, kind="ExternalInput").ap()
    cL_d = nc.dram_tensor("cL_in", [128, COLS * (M + 1)], f32, kind="ExternalInput").ap()
    cB_d = nc.dram_tensor("cB_in", [128, (COLS + 1) * M], f32, kind="ExternalInput").ap()
    bc128_d = nc.dram_tensor("bc128_in", [128, 128], f32, kind="ExternalInput").ap()
    sdn_d = nc.dram_tensor("sdn_in", [128, 128], f32, kind="ExternalInput").ap()
    sup_d = nc.dram_tensor("sup_in", [128, 128], f32, kind="ExternalInput").ap()
    x_d = nc.dram_tensor("x_out", [128, F], f32, kind="ExternalOutput").ap()

    with tile.TileContext(nc) as tc, ExitStack() as ctx:
        sb = ctx.enter_context(tc.tile_pool(name="state", bufs=1))
        ps = ctx.enter_context(tc.tile_pool(name="psum", bufs=1, space="PSUM"))

        r = sb.tile([128, FH], f32)       # halo_lo | center | halo_hi
        p = sb.tile([128, F], f32)
        x = sb.tile([128, F], f32)
        q = sb.tile([128, F], f32)        # A @ p via recurrence
        t0 = sb.tile([128, F], f32)
        t1 = sb.tile([128, F], f32)
        t2 = sb.tile([128, F], f32)
        t3 = sb.tile([128, F], f32)
        cD = sb.tile([128, F], f32)
        cL = sb.tile([128, COLS * (M + 1)], f32)
        cB = sb.tile([128, (COLS + 1) * M], f32)
        bc128 = sb.tile([128, 128], f32)
        sdn = sb.tile([128, 128], f32)
        sup = sb.tile([128, 128], f32)

        pap_part = sb.tile([128, 1], f32)
        rr_part = sb.tile([128, 1], f32)
        gamvec = sb.tile([128, 1], f32)   # per-problem gamma, broadcast
        recg = sb.tile([128, 1], f32)     # 1/gamma_old
        recp = sb.tile([128, 1], f32)     # 1/pAp
        avec = sb.tile([128, 1], f32)
        bvec = sb.tile([128, 1], f32)

        pap_ps = ps.tile([128, 1], f32)
        gam_ps = ps.tile([128, 1], f32)
        hlo_ps = ps.tile([128, M], f32)
        hhi_ps = ps.tile([128, M], f32)

        def v3(ap2d):
            return ap2d.rearrange("p (c j) -> p c j", c=COLS, j=M)

        r_c2 = r[:, M:M + F]
        r_c3 = v3(r_c2)
        r_jm1 = v3(r[:, M - 1:M - 1 + F])
        r_jp1 = v3(r[:, M + 1:M + 1 + F])
        r_km1 = v3(r[:, 0:F])
        r_kp1 = v3(r[:, 2 * M:2 * M + F])
        cL3 = cL[:].rearrange("p (c j) -> p c j", c=COLS, j=M + 1)
        cLl = cL3[:, :, 0:M]
        cLr = cL3[:, :, 1:M + 1]
        cB3 = cB[:].rearrange("p (c j) -> p c j", c=COLS + 1, j=M)
        cBb = cB3[:, 0:COLS, :]
        cBt = cB3[:, 1:COLS + 1, :]
        cD3 = v3(cD[:])

        nc.sync.dma_start(cD[:], cD_d)
        nc.sync.dma_start(cL[:], cL_d)
        nc.sync.dma_start(cB[:], cB_d)
        nc.sync.dma_start(bc128[:], bc128_d)
        nc.sync.dma_start(sdn[:], sdn_d)
        nc.sync.dma_start(sup[:], sup_d)
        nc.sync.dma_start(r_c2, f_d)
        nc.sync.dma_start(p[:], f_d)

        def halo_update():
            nc.tensor.matmul(hlo_ps[:], sdn[:], r[:, F:F + M])
            nc.tensor.matmul(hhi_ps[:], sup[:], r[:, M:2 * M])
            nc.scalar.copy(r[:, 0:M], hlo_ps[:])
            nc.scalar.copy(r[:, F + M:F + 2 * M], hhi_ps[:])

        def stencil_w():
            """t0 = A @ r (j-terms on DVE, k-products on GpSimd)."""
            nc.gpsimd.tensor_mul(v3(t2[:]), cBb, r_km1)
            nc.gpsimd.tensor_mul(v3(t3[:]), cBt, r_kp1)
            nc.vector.tensor_mul(v3(t0[:]), cD3, r_c3)
            nc.vector.tensor_mul(v3(t1[:]), cLl, r_jm1)
            nc.vector.tensor_add(t0[:], t0[:], t1[:])
            nc.vector.tensor_mul(v3(t1[:]), cLr, r_jp1)
            nc.vector.tensor_add(t0[:], t0[:], t1[:])
            nc.vector.tensor_add(t0[:], t0[:], t2[:])
            nc.vector.tensor_add(t0[:], t0[:], t3[:])

        # ---- init: x=0, r=p=f, q = A p, gamma0
        nc.vector.memset(x[:], 0.0)
        halo_update()
        nc.scalar.activation(t1[:], r_c2, Act.Square, accum_out=rr_part[:])
        nc.tensor.matmul(gam_ps[:], bc128[:], rr_part[:])
        nc.scalar.copy(gamvec[:], gam_ps[:])
        stencil_w()
        nc.vector.tensor_copy(q[:], t0[:])

        # ---- 300 CG iterations
        with tc.For_i(0, iters) as _i:
            nc.vector.reciprocal(recg[:], gamvec[:])

            # pAp and alpha
            nc.vector.tensor_mul(t3[:], p[:], q[:])
            nc.scalar.activation(t3[:], t3[:], Act.Copy, accum_out=pap_part[:])
            nc.tensor.matmul(pap_ps, bc128[:], pap_part[:])
            nc.vector.reciprocal(recp[:], pap_ps)
            nc.vector.tensor_mul(avec[:], gamvec[:], recp[:])

            # x += alpha*p (ACT+GpSimd, off critical) ; r -= alpha*q (DVE)
            nc.scalar.activation(t2[:], p[:], Act.Copy, scale=avec[:])
            nc.gpsimd.tensor_add(x[:], x[:], t2[:])
            nc.vector.tensor_scalar_mul(t1[:], q[:], avec[:])
            nc.vector.tensor_sub(r_c2, r_c2, t1[:])
            halo_update()

            # gamma' and beta (hidden under the stencil)
            nc.scalar.activation(t1[:], r_c2, Act.Square, accum_out=rr_part[:])
            nc.tensor.matmul(gam_ps[:], bc128[:], rr_part[:])
            nc.vector.tensor_mul(bvec[:], gam_ps, recg[:])
            nc.scalar.copy(gamvec[:], gam_ps[:])

            # w = A r
            stencil_w()

            # p = r + beta*p (GpSimd) ; q = w + beta*q (DVE)
            nc.gpsimd.tensor_scalar_mul(t2[:], p[:], bvec[:])
            nc.gpsimd.tensor_add(p[:], r_c2, t2[:])
            nc.vector.tensor_scalar_mul(t1[:], q[:], bvec[:])
            nc.vector.tensor_add(q[:], t0[:], t1[:])

        nc.sync.dma_start(x_d, x[:])

    nc.compile()
    return nc


def _build_nc(iters):
    from contextlib import ExitStack
    import concourse.bass as bass
    import concourse.tile as tile
    from concourse import bacc, mybir

    f32 = mybir.dt.float32
    Alu = mybir.AluOpType
    Act = mybir.ActivationFunctionType

    nc = bacc.Bacc("TRN2", target_bir_lowering=False, debug=False)

    f_d = nc.dram_tensor("f_in", [128, F], f32, kind="ExternalInput").ap()
    cD_d = nc.dram_tensor("cD_in", [128, F], f32, kind="ExternalInput").ap()
    cL_d = nc.dram_tensor("cL_in", [128, COLS * (M + 1)], f32, kind="ExternalInput").ap()
    cB_d = nc.dram_tensor("cB_in", [128, (COLS + 1) * M], f32, kind="ExternalInput").ap()
    bc128_d = nc.dram_tensor("bc128_in", [128, 128], f32, kind="ExternalInput").ap()
    sdn_d = nc.dram_tensor("sdn_in", [128, 128], f32, kind="ExternalInput").ap()
    sup_d = nc.dram_tensor("sup_in", [128, 128], f32, kind="ExternalInput").ap()
    x_d = nc.dram_tensor("x_out", [128, F], f32, kind="ExternalOutput").ap()

    with tile.TileContext(nc) as tc, ExitStack() as ctx:
        sb = ctx.enter_context(tc.tile_pool(name="state", bufs=1))
        ps = ctx.enter_context(tc.tile_pool(name="psum", bufs=1, space="PSUM"))

        p = sb.tile([128, FH], f32)       # halo_lo | center | halo_hi
        r = sb.tile([128, F], f32)
        x = sb.tile([128, F], f32)
        q = sb.tile([128, F], f32)        # A @ p
        t0 = sb.tile([128, F], f32)       # DVE stencil accumulator
        t1 = sb.tile([128, F], f32)       # DVE-only scratch (products, axpy terms)
        t2 = sb.tile([128, F], f32)       # GpSimd m3 product / ACT rr junk
        t3 = sb.tile([128, F], f32)       # GpSimd m4 product / pAp product / x term
        t4 = sb.tile([128, F], f32)       # GpSimd m1 product (dedicated)
        cD = sb.tile([128, F], f32)
        cL = sb.tile([128, COLS * (M + 1)], f32)
        cB = sb.tile([128, (COLS + 1) * M], f32)
        bc128 = sb.tile([128, 128], f32)
        sdn = sb.tile([128, 128], f32)
        sup = sb.tile([128, 128], f32)

        pap_part = sb.tile([128, 1], f32)
        rr_part = sb.tile([128, 1], f32)
        gamvec = sb.tile([128, 1], f32)   # per-problem gamma, broadcast
        recg = sb.tile([128, 1], f32)
        recp = sb.tile([128, 1], f32)
        avec = sb.tile([128, 1], f32)
        aneg = sb.tile([128, 1], f32)
        bvec = sb.tile([128, 1], f32)

        pap_ps = ps.tile([128, 1], f32)
        gam_ps = ps.tile([128, 1], f32)
        hlo_ps = ps.tile([128, M], f32)
        hhi_ps = ps.tile([128, M], f32)

        # 3D views [128, 16, 256] over the stencil operands
        def v3(ap2d):
            return ap2d.rearrange("p (c j) -> p c j", c=COLS, j=M)

        p_c2 = p[:, M:M + F]
        p_c3 = v3(p_c2)
        p_jm1 = v3(p[:, M - 1:M - 1 + F])
        p_jp1 = v3(p[:, M + 1:M + 1 + F])
        p_km1 = v3(p[:, 0:F])
        p_kp1 = v3(p[:, 2 * M:2 * M + F])
        cL3 = cL[:].rearrange("p (c j) -> p c j", c=COLS, j=M + 1)
        cLl = cL3[:, :, 0:M]        # multiplies p_jm1
        cLr = cL3[:, :, 1:M + 1]    # multiplies p_jp1 (= K_right view)
        cB3 = cB[:].rearrange("p (c j) -> p c j", c=COLS + 1, j=M)
        cBb = cB3[:, 0:COLS, :]     # multiplies p_km1
        cBt = cB3[:, 1:COLS + 1, :] # multiplies p_kp1 (= K_top view)
        cD3 = v3(cD[:])

        # ---- load inputs
        nc.sync.dma_start(cD[:], cD_d)
        nc.sync.dma_start(cL[:], cL_d)
        nc.sync.dma_start(cB[:], cB_d)
        nc.sync.dma_start(bc128[:], bc128_d)
        nc.sync.dma_start(sdn[:], sdn_d)
        nc.sync.dma_start(sup[:], sup_d)
        nc.sync.dma_start(r[:], f_d)
        nc.sync.dma_start(p_c2, f_d)

        def halo_update():
            # halo_lo[P] = center_last_col[P-1]; halo_hi[P] = center_first_col[P+1]
            nc.tensor.matmul(hlo_ps[:], sdn[:], p[:, F:F + M])
            nc.tensor.matmul(hhi_ps[:], sup[:], p[:, M:2 * M])
            nc.scalar.copy(p[:, 0:M], hlo_ps[:])
            nc.scalar.copy(p[:, F + M:F + 2 * M], hhi_ps[:])

        # ---- init: x=0, gamma0 = per-problem ||f||^2, p halos
        nc.vector.memset(x[:], 0.0)
        halo_update()
        nc.scalar.activation(t1[:], r[:], Act.Square, accum_out=rr_part[:])
        nc.tensor.matmul(gam_ps[:], bc128[:], rr_part[:])
        nc.scalar.copy(gamvec[:], gam_ps[:])

        # ---- 300 CG iterations
        loop_mode = os.environ.get("KERNEL_LOOP", "unroll4")

        def body(_i):
            # 1/gamma_old for beta, overlappable with the stencil
            nc.vector.reciprocal(recg[:], gamvec[:])

            # q = A @ p  (GpSimd: k-shift products; DVE: the rest)
            nc.gpsimd.tensor_mul(v3(t2[:]), cBb, p_km1)
            nc.gpsimd.tensor_mul(v3(t3[:]), cBt, p_kp1)
            nc.vector.tensor_mul(v3(t0[:]), cD3, p_c3)
            nc.vector.tensor_mul(v3(t1[:]), cLl, p_jm1)
            nc.vector.tensor_add(t0[:], t0[:], t1[:])
            nc.vector.tensor_mul(v3(t1[:]), cLr, p_jp1)
            nc.vector.tensor_add(t0[:], t0[:], t1[:])
            nc.vector.tensor_add(t0[:], t0[:], t2[:])
            nc.vector.tensor_add(q[:], t0[:], t3[:])

            # pAp = sum(p*q) fused in one DVE pass; alpha = gamma/pAp
            nc.vector.scalar_tensor_tensor(
                t3[:], p_c2, 1.0, q[:], Alu.mult, Alu.mult,
                accum_out=pap_part[:])
            nc.tensor.matmul(pap_ps, bc128[:], pap_part[:])
            nc.vector.reciprocal(recp[:], pap_ps)
            nc.vector.tensor_mul(avec[:], gamvec[:], recp[:])
            nc.vector.tensor_scalar_mul(aneg[:], avec[:], -1.0)

            # r = (q * -alpha) + r, one pass; x += alpha*p off-critical
            nc.vector.scalar_tensor_tensor(
                r[:], q[:], aneg[:], r[:], Alu.mult, Alu.add)
            nc.scalar.activation(t3[:], p_c2, Act.Copy, scale=avec[:])
            nc.gpsimd.tensor_add(x[:], x[:], t3[:])

            # gamma' = sum(r*r) fused on DVE (no engine hop); beta
            nc.vector.scalar_tensor_tensor(
                t2[:], r[:], 1.0, r[:], Alu.mult, Alu.mult,
                accum_out=rr_part[:])
            nc.tensor.matmul(gam_ps[:], bc128[:], rr_part[:])
            nc.vector.tensor_mul(bvec[:], gam_ps, recg[:])
            nc.scalar.copy(gamvec[:], gam_ps[:])

            # p = (p * beta) + r in one pass, then refresh halos
            nc.vector.scalar_tensor_tensor(
                p_c2, p_c2, bvec[:], r[:], Alu.mult, Alu.add)
            halo_update()

        if loop_mode == "plain":
            with tc.For_i(0, iters) as _i:
                body(_i)
        elif loop_mode == "py":
            for _i in range(iters):
                body(_i)
        elif loop_mode == "stag":
            with tc.For_i(0, iters, staggered_reset=True) as _i:
                body(_i)
        elif loop_mode.startswith("unroll"):
            tc.For_i_unrolled(0, iters, 1, body, max_unroll=int(loop_mode[6:]))
        else:
            raise ValueError(loop_mode)

        nc.sync.dma_start(x_d, x[:])

    nc.compile()
    return nc


VARIANT = os.environ.get("KERNEL_VARIANT", "v3")


def _pack(alpha_core, f_rhs, variant=None):
    variant = variant or VARIANT
    fn = {"v2": _pack_core_v2, "v3": _pack_core_v3}.get(variant, _pack_core)
    return fn(alpha_core, f_rhs)


def _get_nc(iters, variant=None):
    variant = variant or VARIANT
    key = ("nc", iters, variant, os.environ.get("KERNEL_LOOP", "unroll4"),
           os.environ.get("KERNEL2_LOOP", "unroll4"),
           os.environ.get("KERNEL3_MODE", "dve"),
           os.environ.get("KERNEL2_ABLATE", ""))
    if key not in _CACHE:
        builder = {"std": _build_nc, "qrec": _build_nc_qrec,
                   "v2": _build_nc_v2, "v3": _build_nc_v3}[variant]
        _CACHE[key] = builder(iters)
    return _CACHE[key]


def _expected_inputs(nc):
    import concourse.mybir as mybir
    part = nc.partition_id_tensor.name if nc.partition_id_tensor else None
    names = set()
    for alloc in nc.m.functions[0].allocations:
        if isinstance(alloc, mybir.MemoryLocationSet) and alloc.kind == "ExternalInput":
            nm = alloc.memorylocations[0].name
            if nm != part:
                names.add(nm)
    return names


# ------------------------------------------------------------------- runner

def _make_runner(iters, variant=None):
    """Build the 8-core sharded jit once; returns run(in_maps) -> [x_out]*8."""
    import jax
    from jax.sharding import Mesh, PartitionSpec
    from jax.experimental.shard_map import shard_map
    from concourse import bass2jax, mybir

    nc = _get_nc(iters, variant)
    bass2jax.install_neuronx_cc_hook()
    partition_name = nc.partition_id_tensor.name if nc.partition_id_tensor else None
    in_names, out_names, out_avals, zero_outs = [], [], [], []
    for alloc in nc.m.functions[0].allocations:
        if not isinstance(alloc, mybir.MemoryLocationSet):
            continue
        name = alloc.memorylocations[0].name
        if alloc.kind == "ExternalInput":
            if name != partition_name:
                in_names.append(name)
        elif alloc.kind == "ExternalOutput":
            out_names.append(name)
            shape = tuple(alloc.tensor_shape)
            dtype = mybir.dt.np(alloc.dtype)
            out_avals.append(jax.core.ShapedArray(shape, dtype))
            zero_outs.append(np.zeros(shape, dtype))
    n_params = len(in_names)
    all_in = in_names + out_names + ([partition_name] if partition_name else [])

    def _body(*args):
        ops = list(args)
        if partition_name:
            ops.append(bass2jax.partition_id_tensor())
        return tuple(bass2jax._bass_exec_p.bind(
            *ops, out_avals=tuple(out_avals), in_names=tuple(all_in),
            out_names=tuple(out_names), lowering_input_output_aliases=(),
            sim_require_finite=True, sim_require_nnan=True, nc=nc))

    mesh = Mesh(np.asarray(jax.devices()[:NCORES]), ("core",))
    jf = jax.jit(
        shard_map(_body, mesh=mesh,
                  in_specs=(PartitionSpec("core"),) * (n_params + len(out_names)),
                  out_specs=(PartitionSpec("core"),) * len(out_names),
                  check_rep=False),
        donate_argnums=tuple(range(n_params, n_params + len(out_names))),
        keep_unused=True)

    def prepare(in_maps):
        import jax
        concat_in = [np.concatenate([m[nm] for m in in_maps], axis=0)
                     for nm in in_names]
        dev_in = [jax.device_put(a) for a in concat_in]
        jax.block_until_ready(dev_in)
        return dev_in

    def run_dev(dev_in, fetch=True):
        import jax
        zeros = [np.zeros((NCORES * z.shape[0], *z.shape[1:]), z.dtype)
                 for z in zero_outs]
        outs = jf(*dev_in, *zeros)
        if not fetch:
            jax.block_until_ready(outs)
            return None
        if fetch == "all":
            return {nm: np.asarray(o) for nm, o in zip(out_names, outs)}
        xo = np.asarray(outs[out_names.index("x_out")])
        per_core_rows = xo.shape[0] // NCORES
        return [xo[c * per_core_rows:(c + 1) * per_core_rows] for c in range(NCORES)]

    def run(in_maps):
        return run_dev(prepare(in_maps))

    run.prepare = prepare
    run.run_dev = run_dev
    return run


def _get_runner(iters, variant=None):
    variant = variant or VARIANT
    key = ("runner", iters, variant, os.environ.get("KERNEL_LOOP", "unroll4"),
           os.environ.get("KERNEL2_LOOP", "unroll4"),
           os.environ.get("KERNEL3_MODE", "dve"),
           os.environ.get("KERNEL2_ABLATE", ""))
    if key not in _CACHE:
        _CACHE[key] = _make_runner(iters, variant)
    return _CACHE[key]


def _run(in_maps, iters, variant=None):
    return _get_runner(iters, variant)(in_maps)


def kernel(alpha, f_rhs):
    alpha = np.asarray(alpha, np.float32)
    f_rhs = np.asarray(f_rhs, np.float32)
    in_maps = [_pack(alpha[c * BPC:(c + 1) * BPC], f_rhs)
               for c in range(NCORES)]
    try:
        outs = _run(in_maps, ITERS)
    except Exception:
        # a crashed prior session can leave a core wedged; one retry clears it
        outs = _run(in_maps, ITERS)
    return np.concatenate([_from_dev(o) for o in outs], axis=0)

